# revision 49
# baseline (speedup 1.0000x reference)
"""BiLSTM Trainium2 kernel — transposed-domain recurrence.

Problem: B=32, T=512, I=512, H=512 bidirectional LSTM (torch gate order
i,f,g,o; shared weights across directions; backward outputs stacked in
processing order).

Sharding: 8 cores = 2 directions x 4 batch groups of 8 rows. Every core runs
the IDENTICAL program; backward cores get time-reversed x from the host.

Per-core device program (one direction, 8 batch rows), entirely in the
TRANSPOSED domain (partition dim = gate/hidden dim, free dim = batch):

  - gates^T live in PSUM as [128, cid, batch] per step, where cid = 4*gate+m
    indexes 128-row chunks of the 2048 gate dim (gate order i,f,o,g).
  - The recurrent matmul keeps Wh chunks STATIONARY ([K=128, M=128] bf16
    tiles) and streams h^T as the MOVING operand in bf16: cost is
    ap_size=batch rows/matmul — ~20x less PE streaming than moving Wh
    through the PE each step.
  - gx = Wx @ x^T (+ exact-f32 bias) is pre-accumulated INTO the same PSUM
    banks one 16-step window ahead, so the recurrence matmuls just
    accumulate on top and activations read finished gates straight from
    PSUM. No selector matmuls, no gx DRAM round trip, no PE transposes.
  - The 8 batch rows are split into CH independent chains stepped in an
    interleaved order, so one chain's matmuls run inside the other chain's
    ACT/DVE latency gaps. PSUM: per (chain, window) an i|f tile and an o|g
    tile (so PE writes never WAR-block on ACT reads of the other pair);
    CH=2: 4 tiles x 2 windows = 8 banks exactly.
  - Epilogue per chain-step: g matmuls first so ACT can run tanh(g) before
    sigma(i,f,o) (dataflow scheduler picks by readiness); DVE
    c' = sigma(f)*c + sigma(i)*tanh(g); ACT tanh(c'); DVE h^T =
    sigma(o)*tanh(c') written bf16 into an 8-step staging buffer that DMAs
    to DRAM (y is produced transposed; host un-transposes).
"""

import numpy as np
import ml_dtypes

B, T, I, H = 32, 512, 512, 512
G4 = 4 * H
BL = 32                # batch rows per core (all 32; cores split dir x T/4)
CH = 2                 # independent interleaved chains per core
R = BL // CH           # batch rows per chain
WIN = 2                # steps per gx window
WARM = 32              # warm-up steps for non-initial time-quarters
TC = T // 4 + WARM     # per-core steps (sequence-parallel quarters)
NCID = 16              # 128-row chunks of the gate dim

_COMPILED = {}


def _build_program(t_steps: int):
    import concourse.bass as bass
    import concourse.tile as tile
    from concourse import bacc, mybir

    dt = mybir.dt
    f32 = dt.float32
    f32r = dt.float32r
    bf16 = dt.bfloat16
    nw = t_steps // WIN
    nyb = t_steps // 8     # y DMA blocks

    nc = bacc.Bacc("TRN2", target_bir_lowering=False, debug=False)

    # ---- DRAM parameters ----
    # stationary weight tiles, split per k-chunk so the 4 DMAs ride
    # parallel DMA engines and unblock the first matmuls early:
    # whs{k}[kp, cid, m] = W^T_perm[k*128+kp, cid*128+m]
    whs_ds = [nc.declare_dram_parameter(f"whs{k}", [128, 16, 128], bf16,
                                        isOutput=False) for k in range(4)]
    wxs_ds = [nc.declare_dram_parameter(f"wxs{k}", [128, 16, 128], bf16,
                                        isOutput=False) for k in range(4)]
    # bias as K=1 matmuls: [0, cid, p] = b[cid*128+p]; ones rhs [1, WIN*R]
    bias_d = nc.declare_dram_parameter("biasT", [1, 16, 128], bf16, isOutput=False)
    ones_d = nc.declare_dram_parameter("ones1", [1, WIN * R], bf16, isOutput=False)
    # x^T in window layout: [i, t, b]
    xT_d = nc.declare_dram_parameter("xT", [I, t_steps, BL], bf16, isOutput=False)
    # y out per chain, transposed-h layout: [blk, p, slot, m, b]
    y_ds = [nc.declare_dram_parameter(f"y{ch}", [nyb, 128, 8, 4, R],
                                      bf16, isOutput=True)
            for ch in range(CH)]

    sigf = mybir.ActivationFunctionType.Sigmoid
    tanhf = mybir.ActivationFunctionType.Tanh

    with tile.TileContext(nc) as tc:
        with (
            tc.tile_pool(name="const", bufs=1) as const_pool,
            tc.tile_pool(name="xw", bufs=3) as xw_pool,
            tc.tile_pool(name="ep", bufs=2) as ep_pool,
            tc.tile_pool(name="yb", bufs=2) as yb_pool,
            tc.tile_pool(name="win", bufs=2, space="PSUM") as win_pool,
        ):
            # ---- constants ----
            whsk, wxsk = [], []
            for k in range(4):
                t_ = const_pool.tile([128, 16, 128], bf16, tag=f"whs{k}",
                                     name=f"whs{k}")
                nc.sync.dma_start(out=t_, in_=whs_ds[k][:, :, :])
                whsk.append(t_)
            for k in range(4):
                t_ = const_pool.tile([128, 16, 128], bf16, tag=f"wxs{k}",
                                     name=f"wxs{k}")
                nc.sync.dma_start(out=t_, in_=wxs_ds[k][:, :, :])
                wxsk.append(t_)
            biasT = const_pool.tile([1, 16, 128], bf16, tag="biasT")
            nc.sync.dma_start(out=biasT, in_=bias_d[:, :, :])
            ones1 = const_pool.tile([1, WIN * R], bf16, tag="ones1")
            nc.sync.dma_start(out=ones1, in_=ones_d[:, :])

            # ---- x window loads: 4 tiles [128, WIN, BL] per window ----
            xw_tiles = {}

            def load_xw(w):
                tiles = []
                for k in range(4):
                    t_ = xw_pool.tile([128, WIN, BL], bf16, tag=f"xw{k}",
                                      name=f"xw{w}_{k}")
                    nc.sync.dma_start(
                        out=t_,
                        in_=xT_d[k * 128:(k + 1) * 128, w * WIN:(w + 1) * WIN, :],
                    )
                    tiles.append(t_)
                xw_tiles[w] = tiles

            # ---- PSUM gate tiles: per (window, chain): q=0 i|f|o, q=1 g ----
            # cids 0-11 = i,f,o; 12-15 = g.
            win_tiles = {}

            def alloc_win(w):
                win_tiles[w] = [
                    [win_pool.tile([128, 12, WIN * R], f32, tag=f"win{ch}0",
                                   name=f"win{w}_{ch}_0"),
                     win_pool.tile([128, 4, WIN * R], f32, tag=f"win{ch}1",
                                   name=f"win{w}_{ch}_1")]
                    for ch in range(CH)
                ]

            def cid_tile(w, ch, cid):
                if cid < 12:
                    return win_tiles[w][ch][0], cid
                return win_tiles[w][ch][1], cid - 12

            def emit_bias_mm(w, ch, cid):
                # K=1 matmul per cid: streams exactly WIN*R rows. start=True
                # only on the first cid of each tile/bank (pending-zero rule)
                tile_, idx = cid_tile(w, ch, cid)
                nc.tensor.matmul(
                    tile_[:, idx, :],
                    lhsT=biasT[:, cid, :],
                    rhs=ones1[:, :],
                    start=(cid in (0, 12)), stop=False, skip_group_check=True,
                )

            def emit_gx_mm(w, ch, cid, k):
                tile_, idx = cid_tile(w, ch, cid)
                nc.tensor.matmul(
                    tile_[:, idx, :],
                    lhsT=wxsk[k][:, cid, :],
                    rhs=xw_tiles[w][k][:, :, ch * R:(ch + 1) * R],
                    start=False, stop=False, skip_group_check=True,
                )

            # ---- prologue ----
            load_xw(0)
            if nw > 1:
                load_xw(1)
            alloc_win(0)
            for ch in range(CH):
                for cid in range(NCID):
                    emit_bias_mm(0, ch, cid)
            for ch in range(CH):
                for cid in range(NCID):
                    for k in range(4):
                        emit_gx_mm(0, ch, cid, k)

            xs, hTs, ybufs = [], [], []
            for ch in range(CH):
                h0 = const_pool.tile([128, 4, R], bf16, tag=f"h0{ch}",
                                     name=f"h0{ch}")
                nc.vector.memset(h0, 0.0)
                x0 = ep_pool.tile([128, 8, R], f32, tag=f"c{ch}",
                                  name=f"x0{ch}")
                nc.vector.memset(x0[:, 4:8, :], 0.0)
                hTs.append(h0)
                xs.append(x0)
                ybufs.append(None)

            # cid emission order and per-tile last cid for stop flags
            order_g = list(range(12, 16))
            last_in_q = {0: 11, 1: 15}

            # gx backlog queue: (w, ch, cid, k) emitted a few at a time
            gx_queue = []

            def drain_gx(n):
                for _ in range(min(n, len(gx_queue))):
                    emit_gx_mm(*gx_queue.pop(0))

            def housekeeping2(t):
                w, tw = t // WIN, t % WIN
                if w + 1 >= nw:
                    return
                if tw == 0:
                    if w + 2 < nw:
                        load_xw(w + 2)
                    alloc_win(w + 1)
                    for ch in range(CH):
                        for cid in range(NCID):
                            emit_bias_mm(w + 1, ch, cid)
                    for ch in range(CH):
                        for cid in range(NCID):
                            for k in range(4):
                                gx_queue.append((w + 1, ch, cid, k))

            for t in range(t_steps):
                w, tw = t // WIN, t % WIN
                sl = slice(tw * R, (tw + 1) * R)
                stop_ok = tw == WIN - 1
                gx_per_slot = (len(gx_queue) + (WIN - 2) * CH - 1) // max(
                    (WIN - 1 - max(tw, 1)) * CH, 1)

                sifos = []
                for ch in range(CH):
                    qtiles = win_tiles[w][ch]
                    hT = hTs[ch]

                    # g matmuls FIRST: tanh(g) becomes ready before
                    # sigma(ifo), so the ACT runs it first and the ig-path
                    # only gates on sigma(ifo)'s ack
                    for cid in order_g:
                        for k in range(4):
                            nc.tensor.matmul(
                                qtiles[1][:, cid - 12, sl],
                                lhsT=whsk[k][:, cid, :],
                                rhs=hT[:, k, :],
                                start=False,
                                stop=(stop_ok and k == 3
                                      and last_in_q[1] == cid),
                                skip_group_check=True,
                            )
                    nc.scalar.activation(xs[ch][:, 0:4, :],
                                         qtiles[1][:, :, sl], tanhf)
                    # i,f matmuls then sigma(if): emitting sigma(if)
                    # BEFORE the o matmuls keeps them off its tile-granular
                    # dependency; the o matmuls WAR-wait on sigma(if)
                    # instead, which is harmless (sigma(o) feeds only the
                    # tail h-multiply)
                    for cid in range(0, 8):
                        for k in range(4):
                            nc.tensor.matmul(
                                qtiles[0][:, cid, sl],
                                lhsT=whsk[k][:, cid, :],
                                rhs=hT[:, k, :],
                                start=False, stop=False,
                                skip_group_check=True,
                            )
                    sifo = ep_pool.tile([128, 12, R], f32, tag=f"sifo{ch}",
                                        name=f"sifo{ch}")
                    nc.scalar.activation(sifo[:, 0:8, :],
                                         qtiles[0][:, 0:8, sl], sigf)
                    for cid in range(8, 12):
                        for k in range(4):
                            nc.tensor.matmul(
                                qtiles[0][:, cid, sl],
                                lhsT=whsk[k][:, cid, :],
                                rhs=hT[:, k, :],
                                start=False,
                                stop=(stop_ok and k == 3
                                      and last_in_q[0] == cid),
                                skip_group_check=True,
                            )
                    nc.scalar.activation(sifo[:, 8:12, :],
                                         qtiles[0][:, 8:12, sl], sigf)
                    sifos.append(sifo)
                    drain_gx(gx_per_slot)

                cns = []
                for ch in range(CH):
                    # one wide multiply: [sigma_i|sigma_f] * [tg|c] = [ig|fc]
                    figc = ep_pool.tile([128, 8, R], f32, tag=f"fg{ch}",
                                        name=f"figc{ch}")
                    nc.vector.tensor_mul(figc, sifos[ch][:, 0:8, :], xs[ch])
                    xn = ep_pool.tile([128, 8, R], f32, tag=f"c{ch}",
                                      name=f"xn{ch}")
                    nc.vector.tensor_add(xn[:, 4:8, :], figc[:, 0:4, :],
                                         figc[:, 4:8, :])
                    cns.append(xn)
                    xs[ch] = xn

                tcs = []
                for ch in range(CH):
                    tc_t = ep_pool.tile([128, 4, R], f32, tag=f"tanc{ch}",
                                        name=f"tanc{ch}")
                    nc.scalar.activation(tc_t, cns[ch][:, 4:8, :], tanhf)
                    tcs.append(tc_t)

                for ch in range(CH):
                    if t % 8 == 0:
                        ybufs[ch] = yb_pool.tile([128, 8, 4, R], bf16,
                                                 tag=f"yb{ch}",
                                                 name=f"yb{ch}_{t // 8}")
                    hTn = ybufs[ch][:, t % 8, :, :]
                    nc.vector.tensor_mul(hTn, sifos[ch][:, 8:12, :], tcs[ch])
                    hTs[ch] = hTn
                    if t % 8 == 7:
                        nc.sync.dma_start(
                            out=y_ds[ch][t // 8],
                            in_=ybufs[ch],
                        )

                housekeeping2(t)

    nc.compile()
    return nc


def _get_program(t_steps: int):
    # the public key is the FULL sequence length; the device program runs
    # TC = T/2 + WARM steps (each core covers one time-half with warm-up)
    t_core = TC if t_steps == T else t_steps
    if t_core not in _COMPILED:
        _COMPILED[t_core] = _build_program(t_core)
    return _COMPILED[t_core]


# gate permutation [i, f, o, g] from torch order [i, f, g, o]
_PERM = np.concatenate(
    [np.arange(0, 512), np.arange(512, 1024), np.arange(1536, 2048),
     np.arange(1024, 1536)]
)


def _prep_weights(Wx, bx, Wh, bh):
    def stat(Wm):
        # [kp, cid*4+k, m] = W^T_perm[k*128+kp, cid*128+m]
        WT = np.ascontiguousarray(Wm[_PERM].T)  # [512, 2048]
        return np.ascontiguousarray(
            WT.reshape(4, 128, 16, 128).transpose(1, 2, 0, 3).reshape(128, 64, 128)
        )

    whs = stat(Wh).astype(ml_dtypes.bfloat16)
    wxs = stat(Wx).astype(ml_dtypes.bfloat16)
    whsk = [np.ascontiguousarray(whs[:, k::4, :]) for k in range(4)]
    wxsk = [np.ascontiguousarray(wxs[:, k::4, :]) for k in range(4)]
    b = (bx + bh)[_PERM].astype(np.float32)
    biasT = np.ascontiguousarray(b.reshape(1, 16, 128)).astype(ml_dtypes.bfloat16)
    ones1 = np.ones((1, WIN * R), ml_dtypes.bfloat16)
    return whsk, wxsk, biasT, ones1


def _host_prep(x, Wx, bx, Wh, bh, t_steps):
    whsk, wxsk, biasT, ones1 = _prep_weights(Wx, bx, Wh, bh)
    in_maps = []
    if t_steps == T:
        for core in range(8):
            d, s = divmod(core, 4)
            xc = x
            if d == 1:
                xc = xc[:, ::-1]
            lo = max(s * (T // 4) - WARM, 0)
            xc = xc[:, lo:lo + TC]
            xT = np.ascontiguousarray(xc.transpose(2, 1, 0)).astype(
                ml_dtypes.bfloat16)
            in_maps.append({
                **{f"whs{k}": whsk[k] for k in range(4)},
                **{f"wxs{k}": wxsk[k] for k in range(4)},
                "biasT": biasT, "ones1": ones1, "xT": xT,
            })
    else:
        xc = x[:BL, :t_steps]
        xT = np.ascontiguousarray(xc.transpose(2, 1, 0)).astype(
            ml_dtypes.bfloat16)
        in_maps.append({
            **{f"whs{k}": whsk[k] for k in range(4)},
            **{f"wxs{k}": wxsk[k] for k in range(4)},
            "biasT": biasT, "ones1": ones1, "xT": xT,
        })
    return in_maps


def _assemble_y(y):
    # y: [T/8, 128, 8, 4, rows] bf16 -> [T, rows, H] f32
    t8, rows = y.shape[0], y.shape[4]
    return (
        y.astype(np.float32)
        .transpose(0, 2, 4, 3, 1)          # [blk, slot, b, m, p]
        .reshape(t8 * 8, rows, H)
    )


def kernel(x, Wx, bx, Wh, bh):
    from concourse.bass_utils import run_bass_kernel_spmd

    x = np.asarray(x, dtype=np.float32)
    Wx = np.asarray(Wx, dtype=np.float32)
    bx = np.asarray(bx, dtype=np.float32)
    Wh = np.asarray(Wh, dtype=np.float32)
    bh = np.asarray(bh, dtype=np.float32)
    nc = _get_program(T)
    in_maps = _host_prep(x, Wx, bx, Wh, bh, T)
    res = run_bass_kernel_spmd(nc, in_maps, list(range(8)))
    out = np.empty((B, T, 2 * H), dtype=np.float32)
    qt = T // 4
    for core in range(8):
        d, s = divmod(core, 4)
        yh = np.concatenate(
            [_assemble_y(np.asarray(res.results[core][f"y{ch}"]))
             for ch in range(CH)], axis=1)  # [TC, BL, H]
        used = yh[0:qt] if s == 0 else yh[WARM:WARM + qt]
        out[:, s * qt:(s + 1) * qt, d * H:(d + 1) * H] = used.transpose(1, 0, 2)
    return out


def _np_lstm(x, Wx, bx, Wh, bh):
    """Single-direction numpy reference (forward order)."""
    b_, t_, _ = x.shape
    h = np.zeros((b_, H), np.float32)
    c = np.zeros((b_, H), np.float32)
    gx = x @ Wx.T + bx
    ys = []
    for t in range(t_):
        gates = gx[:, t] + h @ Wh.T + bh
        i_g, f_g, g_g, o_g = np.split(gates, 4, axis=1)
        c = c * (1 / (1 + np.exp(-f_g))) + (1 / (1 + np.exp(-i_g))) * np.tanh(g_g)
        h = (1 / (1 + np.exp(-o_g))) * np.tanh(c)
        ys.append(h)
    return np.stack(ys, 1)


def _selftest(t_steps=40):
    from concourse.bass_interp import CoreSim

    rng = np.random.default_rng(0)
    s = 1.0 / np.sqrt(H)
    x = rng.standard_normal((B, T, I), dtype=np.float32)
    Wx = (rng.standard_normal((G4, I)) * s).astype(np.float32)
    bx = (rng.standard_normal(G4) * s).astype(np.float32)
    Wh = (rng.standard_normal((G4, H)) * s).astype(np.float32)
    bh = (rng.standard_normal(G4) * s).astype(np.float32)

    nc = _get_program(t_steps)
    in_maps = _host_prep(x, Wx, bx, Wh, bh, t_steps)
    sim = CoreSim(nc, trace=False)
    for k, v in in_maps[0].items():
        sim.tensor(k)[:] = v
    sim.simulate()
    yh = np.concatenate(
        [_assemble_y(np.array(sim.tensor(f"y{ch}"))) for ch in range(CH)],
        axis=1)  # [t, BL, H]
    ref = _np_lstm(x[:BL, :t_steps], Wx, bx, Wh, bh)
    err = np.abs(yh.transpose(1, 0, 2) - ref)
    scale = np.abs(ref).max()
    print(f"selftest T={t_steps}: max abs err {err.max():.3e} (scale {scale:.3f}) "
          f"rel {err.max() / scale:.3e}")
    return err.max() / scale


if __name__ == "__main__":
    _selftest(40)
